# revision 34
# baseline (speedup 1.0000x reference)
"""MAGNN intra-metapath aggregator on 8 TRN2 NeuronCores.

bf16 streaming design. The kernel is HBM-bandwidth bound (the 819 MB
paths tensor must be streamed once; per-NC HBM share is ~358 GB/s), so
the host casts paths to bf16 before upload — 51.2 MB/core instead of
102.4 MB/core, halving the f32 DMA floor of ~286 us to ~143 us.
Output rel error from the cast is ~1.4e-3, far under the 2e-2 gate.

Per 2 MB chunk (512 instances, 4 per SBUF partition as 4 KB free-dim
blocks, two HBM-consecutive instances per 8 KB DMA descriptor): DVE
folds 16 path nodes to 4-node partials with two flat bf16 adds per
block; PE transposes the partials via bf16 matmuls against a bf16
identity (fp32 PSUM accumulate, 4-deep) into one [128, 512] PSUM tile;
one ACT copy stages repsum^T to SBUF; per-block matmuls add the target
bias and compute head scores; LeakyReLU+exp on DVE/ACT; PE accumulates
alpha-weighted partials and weight sums into persistent PSUM banks.
Per-core partials [H, D+1] are combined on the host (cheaper than a
device AllReduce). Tail (12500 = 24*512 + 212) uses a
1-instance-per-partition path.

Measured on HW via a For_i hardware-loop repeat-differential (the axon
RPC noise is tens of ms, so the differential uses a ~37 ms signal):
~190 us/pass vs the 288 us f32 baseline. An fp8e4 upload variant
(in8=True; SWDGE casting DMA upconverts to bf16 in-flight, verified
bit-exact) is available behind the IN8 flag."""

import numpy as np

from concourse import bacc, masks, mybir, tile
from concourse.bass_utils import run_bass_kernel_spmd

N, L, D, H = 100000, 16, 128, 8
NCORES = 8
NS = N // NCORES            # 12500 instances per core
CHUNK = 128                 # instances per small (tail) tile
BIG = 512                   # instances per big tile (4 per partition)
NB = 4                      # blocks per partition in a big tile
F32 = mybir.dt.float32
BF16 = mybir.dt.bfloat16
F8 = mybir.dt.float8e4
# fp8 upload halves HBM bytes again; the gpsimd (SWDGE) DMA upcasts
# fp8e4 -> bf16 in the DMA datapath, so all on-chip compute stays bf16.
IN8 = False
PATHS_NP_DTYPE = mybir.dt.np(F8 if IN8 else BF16)
AF = mybir.ActivationFunctionType

_cached_nc = None


def _build(ns=NS, repeat=1, hw_loop=1, stage=3, q2=True, in8=IN8, **_compat):
    # stage: timing-ablation level. 0 = DMA only; 1 = +DVE folds;
    # 2 = +scores (PE transposes, e, leakyrelu/exp); 3 = full (+acc).
    # q2: big-chunk DMA packs two HBM-consecutive instances per
    # partition-line piece (8 KB descriptors instead of 4 KB). Instance ->
    # (partition, block) placement changes, which is semantically neutral
    # (every per-instance computation is position-independent).
    nbig = ns // BIG
    tail_chunks = []
    t0 = nbig * BIG
    rem = ns - t0
    while rem > 0:
        cnt = min(CHUNK, rem)
        tail_chunks.append((t0, cnt))
        t0 += cnt
        rem -= cnt
    nc = bacc.Bacc(
        "TRN2",
        target_bir_lowering=False,
        debug=False,
        enable_asserts=False,
        num_devices=NCORES,
    )
    paths_d = nc.dram_tensor(
        "paths", [ns, L, D], F8 if in8 else BF16, kind="ExternalInput"
    )
    tgt_d = nc.dram_tensor("target_feat", [D], F32, kind="ExternalInput")
    af_d = nc.dram_tensor("attn_fc", [H, 2 * D], F32, kind="ExternalInput")
    out_d = nc.dram_tensor("out", [H * (D + 1)], F32, kind="ExternalOutput")

    FD = L * D  # 2048 elements per instance

    with tile.TileContext(nc) as tc:
        with (
            tc.tile_pool(name="const", bufs=1) as constp,
            tc.tile_pool(name="inp", bufs=6) as inp,
            tc.tile_pool(name="work", bufs=4) as work,
            tc.tile_pool(name="ps", bufs=1, space="PSUM") as psp,
        ):
            # ---------- constants ----------
            ident = constp.tile([128, 128], F32)
            masks.make_identity(nc, ident[:])
            ident16 = constp.tile([128, 128], BF16)
            nc.vector.tensor_copy(ident16[:], ident[:])
            af = constp.tile([H, 2 * D], F32)
            nc.sync.dma_start(af[:], af_d.ap())
            tf = constp.tile([D, 1], F32)
            nc.sync.dma_start(tf[:], tgt_d.ap().rearrange("(d one) -> d one", one=1))
            ones_row = constp.tile([1, CHUNK], F32)
            nc.vector.memset(ones_row[:], 1.0)
            ones_col = constp.tile([CHUNK, 1], BF16)
            nc.vector.memset(ones_col[:], 1.0)

            # setup PSUM tiles share the "pt" tag/bufs with the main loop
            # (setup runs once, before the loop) to stay within 8 banks
            # a_tT [D, H]: transpose of attn_fc[:, :D] (f32, setup only)
            ps_t = psp.tile([128, CHUNK], F32, tag="pt", bufs=2)
            a_tT = constp.tile([D, H], F32)
            nc.tensor.transpose(ps_t[:D, :H], af[:H, 0:D], ident[:H, :H])
            nc.vector.tensor_copy(a_tT[:], ps_t[:D, :H])
            # a_rT [D, H] scaled by 1/16 (folds the path-mean into scores),
            # in bf16 so it can pair with the bf16 repsum^T stationary
            ps_r = psp.tile([128, CHUNK], F32, tag="pt", bufs=2)
            a_rT = constp.tile([D, H], BF16)
            nc.tensor.transpose(ps_r[:D, :H], af[:H, D : 2 * D], ident[:H, :H])
            nc.scalar.mul(a_rT[:], ps_r[:D, :H], 1.0 / L)
            # per-head bias b[h] = a_t[h] . target  -> kept as a [1, H] row
            ps_b = psp.tile([128, CHUNK], F32, tag="pt", bufs=2)
            b_row = constp.tile([1, H], F32)
            nc.tensor.matmul(ps_b[:1, :H], tf[:, :1], a_tT[:, :H])
            nc.vector.tensor_copy(b_row[:], ps_b[:1, :H])

            # ---------- persistent accumulators ----------
            acc_p = psp.tile([H, 512], F32, tag="accP")  # 4-node partials
            acc_s = psp.tile([H, 1], F32, tag="accS")    # sum_n w[n,h]

            paths2d = paths_d.ap().rearrange("n l d -> n (l d)")

            # ---------- main streaming loop ----------
            started = [False]

            def scores_from_tmp(tmp, nblk, cnt, e_ps, rT):
                # repsum^T [D, nblk*cnt] via 4-deep accumulating transposes
                # of the 4-node partials, one big ACT copy PSUM->SBUF, then
                # per-block e matmuls.
                pt = psp.tile([128, NB * CHUNK], F32, tag="pt", bufs=2)
                for b in range(nblk):
                    for j in range(4):
                        nc.tensor.matmul(
                            pt[:D, b * CHUNK : b * CHUNK + cnt],
                            tmp[:cnt, b * 1024 + j * D : b * 1024 + (j + 1) * D],
                            ident16[:cnt, :cnt],
                            start=(j == 0), stop=(j == 3),
                            skip_group_check=True,
                        )
                nc.scalar.copy(
                    rT[:, : nblk * CHUNK], pt[:D, : nblk * CHUNK]
                )
                for b in range(nblk):
                    e_dst = e_ps[:cnt, b * H : (b + 1) * H]
                    nc.tensor.matmul(
                        e_dst, ones_row[:1, :cnt], b_row[:1, :],
                        start=True, stop=False, skip_group_check=True,
                    )
                    nc.tensor.matmul(
                        e_dst,
                        rT[:, b * CHUNK : b * CHUNK + cnt],
                        a_rT[:, :],
                        start=False, stop=True, skip_group_check=True,
                    )

            def weights_from_scores(e_ps, wT, cnt, w):
                # LeakyReLU(0.2) then exp; wT in bf16 for the acc matmuls
                sc = work.tile([128, NB * H], F32, tag="sc")
                nc.vector.tensor_scalar_mul(sc[:cnt, :w], e_ps[:cnt, :w], 0.2)
                lr = work.tile([128, NB * H], F32, tag="lr")
                nc.vector.tensor_max(lr[:cnt, :w], sc[:cnt, :w], e_ps[:cnt, :w])
                nc.scalar.activation(wT[:cnt, :w], lr[:cnt, :w], AF.Exp)

            def acc_block(wcol, tmp, off, cnt, first, last):
                nc.tensor.matmul(
                    acc_p[:H, :], wcol, tmp[:cnt, off : off + 512],
                    start=first, stop=last,
                )
                nc.tensor.matmul(
                    acc_s[:H, :], wcol, ones_col[:cnt, :],
                    start=first, stop=last,
                )

            def do_small(n0, cnt, last):
                first = not started[0]
                started[0] = True
                t = inp.tile([128, FD], BF16, tag="in", padded_shape=[128, NB * FD])
                dma_eng = nc.gpsimd if in8 else nc.sync
                dma_eng.dma_start(t[:cnt, :], paths2d[n0 : n0 + cnt, :])
                if stage < 1:
                    return
                tmp = work.tile(
                    [128, 1024], BF16, tag="tree", padded_shape=[128, NB * 1024]
                )
                nc.vector.tensor_add(
                    tmp[:cnt, :], t[:cnt, 0:1024], t[:cnt, 1024:2048]
                )
                nc.vector.tensor_add(
                    tmp[:cnt, 0:512], tmp[:cnt, 0:512], tmp[:cnt, 512:1024]
                )
                if stage < 2:
                    return
                rT = work.tile([D, CHUNK], BF16, tag="rT", padded_shape=[D, NB * CHUNK])
                e_ps = psp.tile(
                    [128, H], F32, tag="e", bufs=2, padded_shape=[128, NB * H]
                )
                scores_from_tmp(tmp, 1, cnt, e_ps, rT)
                wT = work.tile([128, H], BF16, tag="wT", padded_shape=[128, NB * H])
                weights_from_scores(e_ps, wT, cnt, H)
                if stage < 3:
                    return
                acc_block(wT[:cnt, :H], tmp, 0, cnt, first, last)

            def do_big(n0, last):
                first = not started[0]
                started[0] = True
                t = inp.tile([128, NB * FD], BF16, tag="in")
                t3 = t.rearrange("p (b f) -> p b f", b=NB)
                dma_eng = nc.gpsimd if in8 else nc.sync
                if q2:
                    t4 = t.rearrange("p (b q f) -> p b q f", b=NB // 2, q=2)
                    dma_eng.dma_start(
                        t4[:, :, :, :],
                        paths2d[n0 : n0 + BIG, :].rearrange(
                            "(b p q) f -> p b q f", b=NB // 2, q=2
                        ),
                    )
                else:
                    dma_eng.dma_start(
                        t3[:, :, :],
                        paths2d[n0 : n0 + BIG, :].rearrange(
                            "(b p) f -> p b f", b=NB
                        ),
                    )
                if stage < 1:
                    return
                tmp = work.tile([128, NB * 1024], BF16, tag="tree")
                tmp3 = tmp.rearrange("p (b x) -> p b x", b=NB)
                # two fold levels on DVE (16 -> 4 nodes), flat 2D slices
                for b in range(NB):
                    nc.vector.tensor_add(
                        tmp3[:, b, :], t3[:, b, 0:1024], t3[:, b, 1024:2048]
                    )
                for b in range(NB):
                    nc.vector.tensor_add(
                        tmp3[:, b, 0:512],
                        tmp3[:, b, 0:512],
                        tmp3[:, b, 512:1024],
                    )
                if stage < 2:
                    return
                e_ps = psp.tile([128, NB * H], F32, tag="e", bufs=2)
                rT = work.tile([D, NB * CHUNK], BF16, tag="rT")
                scores_from_tmp(tmp, NB, CHUNK, e_ps, rT)
                wT = work.tile([128, NB * H], BF16, tag="wT")
                weights_from_scores(e_ps, wT, 128, NB * H)
                if stage < 3:
                    return
                for b in range(NB):
                    acc_block(
                        wT[:, b * H : (b + 1) * H],
                        tmp,
                        b * 1024,
                        128,
                        first and b == 0,
                        last and b == NB - 1,
                    )

            # repeat>1 / hw_loop>1 are timing-only modes (re-stream the same
            # shard; output then over-counts, never used for correctness
            # runs). hw_loop wraps the repeat passes in a For_i hardware
            # loop so device time scales without instruction-count blowup.
            import contextlib

            loop_ctx = (
                tc.For_i(0, hw_loop) if hw_loop > 1 else contextlib.nullcontext()
            )
            with loop_ctx:
                for r in range(repeat):
                    lr_ = r == repeat - 1
                    for c in range(nbig):
                        do_big(
                            c * BIG,
                            lr_ and not tail_chunks and c == nbig - 1,
                        )
                    for i, (n0, cnt) in enumerate(tail_chunks):
                        do_small(n0, cnt, lr_ and i == len(tail_chunks) - 1)
                started[0] = False

            # ---------- emit per-core partial [p_raw | s] ----------
            # Cross-core combine + softmax normalization happens on the host
            # in kernel(): cheaper than a device AllReduce + bounce trips.
            part = work.tile([H, D + 1], F32, tag="part")
            if stage >= 3:
                # fold the 4-block accumulator 512 -> 128 on DVE
                accs = work.tile([H, 512], F32, tag="accs")
                nc.vector.tensor_copy(accs[:H, :], acc_p[:H, :])
                fold = work.tile([H, 256], F32, tag="fold")
                nc.vector.tensor_add(
                    fold[:H, :], accs[:H, 0:256], accs[:H, 256:512]
                )
                nc.vector.tensor_add(
                    part[:H, 0:D], fold[:H, 0:D], fold[:H, D : 2 * D]
                )
                nc.vector.tensor_copy(part[:H, D : D + 1], acc_s[:H, :])
            else:
                nc.vector.memset(part[:], 0.0)
            nc.sync.dma_start(
                out_d.ap().rearrange("(h d) -> h d", d=D + 1), part[:]
            )

    nc.compile()
    return nc


def kernel(target_feat, paths, attn_fc, **_unused):
    global _cached_nc
    if _cached_nc is None:
        _cached_nc = _build()
    nc = _cached_nc

    paths = np.ascontiguousarray(
        np.asarray(paths, dtype=np.float32).astype(PATHS_NP_DTYPE)
    )
    shards = paths.reshape(NCORES, NS, L, D)
    tgt = np.ascontiguousarray(np.asarray(target_feat, dtype=np.float32))
    af = np.ascontiguousarray(np.asarray(attn_fc, dtype=np.float32))
    in_maps = [
        {"paths": shards[i], "target_feat": tgt, "attn_fc": af}
        for i in range(NCORES)
    ]
    res = run_bass_kernel_spmd(nc, in_maps, core_ids=list(range(NCORES)))
    # host-side combine of the 8 per-core partials [8, D+1]
    tot = np.zeros((H, D + 1), dtype=np.float64)
    for i in range(NCORES):
        tot += np.asarray(res.results[i]["out"], dtype=np.float64).reshape(
            H, D + 1
        )
    out = tot[:, :D] / (L * tot[:, D:])
    return np.ascontiguousarray(out.reshape(H * D).astype(np.float32))
